# revision 1
# baseline (speedup 1.0000x reference)
"""CapsuleLayer (dynamic routing) Trainium2 kernel.

Problem: x[256,1152,8], W[1152,10,8,16] ->
  u_hat = einsum('bik,ijkd->bijd', x, W); 3 routing iterations -> out [256,10,16]

Strategy (8 cores, data-parallel over batch, W replicated on every core):
  Per core: 32 samples. u_hat (5.9M elems) lives entirely in SBUF as bf16 with
  layout [partition = (i%4)*32 + b, free = (I=i//4 in 0..288, d, j)].
  - phase 1: u_hat via 288 block-diagonal matmuls: lhsT = x-blockdiag
    [K=32=(4i x 8k), M=128=(4i x 32b)], rhs = W chunk [32, 160(d,j)], across
    4 PE row-group windows; PSUM evacuated to bf16 SBUF in 3-block groups
    alternating between DVE and ACT (the phase-1 wall).
  - s0 = 0.1*sum_i u_hat via one K=9216 PSUM-accumulated matmul over (i,k)-major
    x/W copies, interleaved with phase-1 matmul groups.
  - routing iterations (software-pipelined in 12 chunks of 24 I-blocks, the
    back stage lagging one chunk):
      front(q): logits r += sum_d u*v  -> A-mul u*vrep (DVE, 2x bf16),
                d-tree levels 16->8->4 on DVE, 4->2->1 + r update on GPSIMD,
                exp on ACT (exp_and_friends act table only; no table swaps).
      back(q):  Z=sum_j exp, 1/Z (DVE); c = e*Zr (GPSIMD); B-mul c*u (DVE);
                s = sum_i via 24 PE matmuls with a static 0/1 partition-folding
                lhsT (sums the 4 i-subgroups; accumulates over I in PSUM).
  - squash on [32,160] tensors: Newton rsqrt via bitcast seed (avoids the
    Sqrt act table), v broadcast to 128 partitions via a tiny PE matmul.
  Cost-model (TimelineSim) estimate: ~225 us/core; measured rel err 2.8e-3.
"""

import sys

if "/opt/trn_rl_repo" not in sys.path:
    sys.path.insert(0, "/opt/trn_rl_repo")

import numpy as np
import ml_dtypes

BF16 = ml_dtypes.bfloat16

B, IC, ID, OC, OD = 256, 1152, 8, 10, 16
NCORES = 8
BC = B // NCORES  # 32 samples per core
NB = IC // 4  # 288 blocks of 4 input capsules
NW, WB = 4, NB // 4  # 4 windows x 72 blocks
NCHUNK, CB = 12, NB // 12  # routing chunks
JD = OC * OD  # 160, stored (d, j): offset = d*OC + j
KQ = IC * ID // 128  # 72 chunks of the 9216 contraction
EPS = 1e-7

_CACHE = {}


def _build_nc():
    from contextlib import ExitStack

    import concourse.bass as bass
    import concourse.tile as tile
    from concourse import bacc, mybir

    dt = mybir.dt
    f32 = dt.float32
    bf = dt.bfloat16
    i32 = dt.int32
    X = mybir.AxisListType.X
    AF = mybir.ActivationFunctionType
    OP = mybir.AluOpType

    nc = bacc.Bacc("TRN2", target_bir_lowering=False, debug=False, num_devices=NCORES)

    d_xblk = nc.dram_tensor("xblk", [128, WB, 128], bf, kind="ExternalInput")
    d_wrhs = nc.dram_tensor("wrhs", [128, WB, JD], bf, kind="ExternalInput")
    d_w9 = nc.dram_tensor("w9", [128, KQ, JD], bf, kind="ExternalInput")
    d_xt9 = nc.dram_tensor("xt9", [128, KQ, BC], bf, kind="ExternalInput")
    d_ones = nc.dram_tensor("onesb", [128, BC], bf, kind="ExternalInput")
    d_bcw = nc.dram_tensor("bcw", [BC, 128], bf, kind="ExternalInput")
    d_y = nc.dram_tensor("y", [BC, OC, OD], f32, kind="ExternalOutput")

    def ap_of(t, free_pairs, extra_off=0):
        """View tile t with custom free-dim [step, count] pairs (partition dim kept)."""
        base = t[:]
        return bass.AP(
            tensor=base.tensor,
            offset=base.offset + extra_off,
            ap=[base.ap[0]] + free_pairs,
        )

    with ExitStack() as ctx:
        tc = ctx.enter_context(tile.TileContext(nc))
        statics = ctx.enter_context(tc.tile_pool(name="statics", bufs=1))
        # w9/xt9 die after s0; share their slot with routing scratch via tag
        shared = ctx.enter_context(tc.tile_pool(name="shared", bufs=1))
        scratch = ctx.enter_context(tc.tile_pool(name="scratch", bufs=4))
        bigs = ctx.enter_context(tc.tile_pool(name="bigs", bufs=1))
        smalls = ctx.enter_context(tc.tile_pool(name="smalls", bufs=2))
        sm2 = ctx.enter_context(tc.tile_pool(name="sm2", bufs=3))
        pe_pool = ctx.enter_context(tc.tile_pool(name="pe", bufs=5, space="PSUM"))
        ps_pool = ctx.enter_context(tc.tile_pool(name="ps", bufs=2, space="PSUM"))
        pv_pool = ctx.enter_context(tc.tile_pool(name="pv", bufs=1, space="PSUM"))

        # ---------- statics in ----------
        xblk = statics.tile([128, WB, 128], bf)
        wrhs = statics.tile([128, WB, JD], bf)
        onesb = statics.tile([128, BC], bf)
        bcw = statics.tile([BC, 128], bf)
        w9 = shared.tile([128, KQ, JD], bf, tag="sc")
        xt9 = statics.tile([128, KQ, BC], bf)
        for w in range(NW):
            sl = slice(32 * w, 32 * w + 32)
            if w == 0:
                third = WB // 3
                for c0 in range(0, WB, third):
                    cs = slice(c0, c0 + third)
                    nc.sync.dma_start(xblk[sl, cs], d_xblk[sl, cs])
                    nc.sync.dma_start(wrhs[sl, cs], d_wrhs[sl, cs])
            else:
                nc.sync.dma_start(xblk[sl], d_xblk[sl])
                nc.sync.dma_start(wrhs[sl], d_wrhs[sl])
        nc.sync.dma_start(w9[:], d_w9[:])
        nc.sync.dma_start(xt9[:], d_xt9[:])
        nc.sync.dma_start(onesb[:], d_ones[:])
        nc.sync.dma_start(bcw[:], d_bcw[:])

        # ---------- persistent big tensors ----------
        uhat = bigs.tile([128, NB, JD], bf)  # free (I, d, j)
        r_bf = bigs.tile([128, NB, OC], bf)  # logits
        vrep = bigs.tile([128, JD], bf, tag="vrep")

        # ---------- phase 1: u_hat (+ s0 matmuls interleaved) ----------
        # s0 = (1/10) sum_i u_hat via one K=9216 accumulated matmul, spread
        # across the phase-1 groups so it finishes with phase 1.
        s0p = ps_pool.tile([BC, JD], f32, tag="sacc")
        for g3 in range(NB // 3):
            pt = pe_pool.tile([128, 3, JD], f32)
            for s in range(3):
                I = 3 * g3 + s
                w, step = divmod(I, WB)
                sl = slice(32 * w, 32 * w + 32)
                nc.tensor.matmul(
                    pt[:, s, :],
                    xblk[sl, step, :],
                    wrhs[sl, step, :],
                    start=True,
                    stop=True,
                    tile_position=(32 * w, 0),
                )
            if 8 <= g3 < 8 + KQ:
                q = g3 - 8
                nc.tensor.matmul(
                    s0p[:],
                    xt9[:, q, :],
                    w9[:, q, :],
                    start=(q == 0),
                    stop=(q == KQ - 1),
                )
            eng = nc.vector if g3 % 2 == 0 else nc.scalar
            if eng is nc.vector:
                nc.vector.tensor_copy(uhat[:, 3 * g3 : 3 * g3 + 3, :], pt[:])
            else:
                nc.scalar.copy(uhat[:, 3 * g3 : 3 * g3 + 3, :], pt[:])

        # ---------- squash helper ----------
        nw_magic = smalls.tile([BC, OC], i32, tag="sq_magic")
        nc.vector.memset(nw_magic[:], 0x5F3759DF)
        nw_onei = smalls.tile([BC, OC], i32, tag="sq_onei")
        nc.vector.memset(nw_onei[:], 1)

        def squash(s_psum, pre_scale):
            """v = squash(pre_scale * s_psum); returns (v_f32, v_bf16) [BC, JD]."""
            s = smalls.tile([BC, JD], f32, tag="sq_s")
            nc.vector.tensor_scalar_mul(s[:], s_psum[:], pre_scale)
            sq = smalls.tile([BC, JD], f32, tag="sq_sq")
            nc.vector.tensor_mul(sq[:], s[:], s[:])
            # sum over d: view (j outer, d inner)
            S = smalls.tile([BC, OC], f32, tag="sq_S")
            sq_v = ap_of(sq, [[1, OC], [OC, OD]])
            nc.vector.reduce_sum(S[:], sq_v, axis=X)
            Sp = smalls.tile([BC, OC], f32, tag="sq_Sp")
            nc.vector.tensor_scalar_add(Sp[:], S[:], EPS)
            # Newton rsqrt of Sp
            half = smalls.tile([BC, OC], i32, tag="sq_half")
            nc.vector.tensor_tensor(
                half[:], Sp[:].bitcast(i32), nw_onei[:], op=OP.arith_shift_right
            )
            y = smalls.tile([BC, OC], f32, tag="sq_y")
            nc.vector.tensor_tensor(
                y[:].bitcast(i32), nw_magic[:], half[:], op=OP.subtract
            )
            for it in range(2):
                t0 = smalls.tile([BC, OC], f32, tag="sq_t0")
                nc.vector.tensor_mul(t0[:], y[:], y[:])
                t1 = smalls.tile([BC, OC], f32, tag="sq_t1")
                nc.vector.tensor_mul(t1[:], t0[:], Sp[:])
                t2 = smalls.tile([BC, OC], f32, tag="sq_t2")
                nc.vector.tensor_scalar(t2[:], t1[:], -0.5, 1.5, op0=OP.mult, op1=OP.add)
                y2 = smalls.tile([BC, OC], f32, tag="sq_y")
                nc.vector.tensor_mul(y2[:], y[:], t2[:])
                y = y2
            # scale = S/(1+S) * rsqrt(S+eps)
            g = smalls.tile([BC, OC], f32, tag="sq_g")
            nc.vector.tensor_scalar_add(g[:], S[:], 1.0)
            gr = smalls.tile([BC, OC], f32, tag="sq_gr")
            nc.vector.reciprocal(gr[:], g[:])
            sc = smalls.tile([BC, OC], f32, tag="sq_sc")
            nc.vector.tensor_mul(sc[:], S[:], gr[:])
            sc2 = smalls.tile([BC, OC], f32, tag="sq_sc2")
            nc.vector.tensor_mul(sc2[:], sc[:], y[:])
            # v = s * scale (broadcast over d); layout (d, j)
            v = smalls.tile([BC, JD], f32, tag="sq_v")
            v_3d = ap_of(v, [[1, OC], [OC, OD]])
            s_3d = ap_of(s, [[1, OC], [OC, OD]])
            sc_3d = ap_of(sc2, [[1, OC], [0, OD]])
            nc.vector.tensor_mul(v_3d, s_3d, sc_3d)
            v_b = smalls.tile([BC, JD], bf, tag="sq_vb")
            nc.vector.tensor_copy(v_b[:], v[:])
            return v, v_b

        def make_vrep(v_b):
            vp = pv_pool.tile([128, JD], f32)
            nc.tensor.matmul(vp[:], bcw[:], v_b[:], start=True, stop=True)
            nc.vector.tensor_copy(vrep[:], vp[:])

        v_f, v_b = squash(s0p, 0.1)
        make_vrep(v_b)

        # ---------- routing iterations ----------
        # Software-pipelined: front(q) = A-mul (DVE) + d-tree/r (GPSIMD) + exp
        # (ACT); back(q) = Z/recip (DVE), c (GPSIMD), B-mul (DVE), PE matmuls.
        # back(q) is emitted two cycles after front(q) so no engine ever waits
        # on another engine's same-cycle output.
        for t in (1, 2):
            sacc = ps_pool.tile([BC, JD], f32, tag="sacc")
            st = {}

            def front(q, t=t):
                I0 = q * CB
                usl = uhat[:, I0 : I0 + CB, :]  # [128, CB, JD]
                t1 = scratch.tile([128, CB, JD], bf, tag="sc")
                vr_b = ap_of(vrep, [[0, CB], [1, JD]])
                nc.vector.tensor_mul(t1[:], usl, vr_b)
                t1v = t1[:].rearrange("p c (d j) -> p c d j", d=OD)
                h = OD // 2
                while h >= 1:
                    tre = nc.vector if h >= 4 else nc.gpsimd
                    tre.tensor_add(
                        t1v[:, :, 0:h, :], t1v[:, :, 0:h, :], t1v[:, :, h : 2 * h, :]
                    )
                    h //= 2
                rsl = r_bf[:, I0 : I0 + CB, :]
                if t == 1:
                    nc.gpsimd.tensor_copy(rsl, t1v[:, :, 0, :])
                else:
                    nc.gpsimd.tensor_add(rsl, rsl, t1v[:, :, 0, :])
                e_t = sm2.tile([128, CB, OC], bf, tag="e")
                nc.scalar.activation(e_t[:], rsl, AF.Exp)
                st[q] = e_t

            def back(q):
                I0 = q * CB
                usl = uhat[:, I0 : I0 + CB, :]
                e_t = st.pop(q)
                z_t = sm2.tile([128, CB], f32, tag="z")
                nc.vector.reduce_sum(z_t[:], e_t[:], axis=X)
                nc.vector.reciprocal(z_t[:], z_t[:])
                c_t = sm2.tile([128, CB, OC], bf, tag="c")
                z_b = ap_of(z_t, [[1, CB], [0, OC]])
                nc.gpsimd.tensor_mul(c_t[:], e_t[:], z_b)
                t2 = scratch.tile([128, CB, JD], bf, tag="sc")
                u4 = usl.rearrange("p c (d j) -> p c d j", d=OD)
                t24 = t2[:].rearrange("p c (d j) -> p c d j", d=OD)
                c_b = ap_of(c_t, [[OC, CB], [0, OD], [1, OC]])
                nc.vector.tensor_mul(t24, u4, c_b)
                for s in range(CB):
                    gi = I0 + s
                    nc.tensor.matmul(
                        sacc[:],
                        onesb[:],
                        t2[:, s, :],
                        start=(gi == 0),
                        stop=(gi == NB - 1),
                    )

            LAG = 1
            for q in range(NCHUNK + LAG):
                if q < NCHUNK:
                    front(q)
                if q >= LAG:
                    back(q - LAG)
            v_f, v_b = squash(sacc, 1.0)
            if t < 2:
                make_vrep(v_b)

        # ---------- output ----------
        stage = smalls.tile([BC, JD], f32, tag="stage")
        st_v = ap_of(stage, [[OD, OC], [1, OD]])  # (j outer, d inner) dense
        vf_v = ap_of(v_f, [[1, OC], [OC, OD]])
        nc.vector.tensor_copy(st_v, vf_v)
        nc.sync.dma_start(d_y[:], stage[:].rearrange("p (j d) -> p j d", j=OC))

    nc.compile()
    return nc


def _prep_host(x, W):
    """Build per-core input maps. x [256,1152,8] f32, W [1152,10,8,16] f32."""
    # W-derived statics (shared across cores)
    Wv = W.reshape(NW, WB, 4, OC, ID, OD)  # [w, step, ip, j, k, d]
    wrhs = np.ascontiguousarray(
        Wv.transpose(0, 2, 4, 1, 5, 3).reshape(128, WB, JD)
    ).astype(BF16)
    Wf = W.transpose(0, 2, 3, 1).reshape(IC * ID, OD, OC)  # [(i,k), d, j]
    w9 = np.ascontiguousarray(
        Wf.reshape(KQ, 128, OD, OC).transpose(1, 0, 2, 3).reshape(128, KQ, JD)
    ).astype(BF16)
    onesb = np.zeros((128, BC), dtype=BF16)
    onesb[np.arange(128), np.arange(128) % 32] = 1
    bcw = np.zeros((BC, 128), dtype=BF16)
    bcw[np.arange(128) % 32, np.arange(128)] = 1

    in_maps = []
    for c in range(NCORES):
        xc = x[c * BC : (c + 1) * BC]  # [32, 1152, 8]
        xv = xc.reshape(BC, NW, WB, 4, ID)  # [b, w, step, ip, k]
        xa = np.zeros((NW, 4, ID, WB, 4, BC), dtype=BF16)  # [w, ip, k, step, ipp, b]
        for ip in range(4):
            xa[:, ip, :, :, ip, :] = xv[:, :, :, ip, :].transpose(1, 3, 2, 0)
        xblk = xa.reshape(128, WB, 128)
        xf = xc.transpose(1, 2, 0).reshape(IC * ID, BC)  # [(i,k), b]
        xt9 = np.ascontiguousarray(
            xf.reshape(KQ, 128, BC).transpose(1, 0, 2)
        ).astype(BF16)
        in_maps.append(
            {
                "xblk": xblk,
                "wrhs": wrhs,
                "w9": w9,
                "xt9": xt9,
                "onesb": onesb,
                "bcw": bcw,
            }
        )
    return in_maps


def kernel(x, W, _trace=False, _trace_kwargs=None):
    from concourse.bass_utils import run_bass_kernel_spmd

    x = np.asarray(x, dtype=np.float32)
    W = np.asarray(W, dtype=np.float32)
    if "nc" not in _CACHE:
        _CACHE["nc"] = _build_nc()
    nc = _CACHE["nc"]
    in_maps = _prep_host(x, W)
    res = run_bass_kernel_spmd(
        nc,
        in_maps,
        core_ids=list(range(NCORES)),
        trace=_trace,
        **(_trace_kwargs or {}),
    )
    _CACHE["last_results"] = res
    out = np.concatenate([res.results[c]["y"] for c in range(NCORES)], axis=0)
    return out

